# revision 9
# baseline (speedup 1.0000x reference)
"""Cross-attention kernel for Trainium2 (8 NeuronCores, SPMD).

Problem: B=4, Nq=1024, Nk=2048, D=512, 8 heads x 64 head-dim, fp32,
full-tensor bias added to scores before softmax.

Sharding: (batch, query-half) -> 8 disjoint shards, one per core. Each core
computes its own (512, 512) slice of the output; no collectives needed.
K/V projections are computed redundantly on the two cores sharing a batch.

Device layout trick: all attention tensors are kept transposed
(feature/key dim on partitions) so every matmul contraction lands on the
partition axis:
  QT[d, q] = (SCALE*Wq) @ xT          (lhsT = wqT chunk, rhs = xT)
  KT[d, k] = Wk @ ctxT
  V[k, i]  = ctxT.T @ Wv.T            (lhsT = ctxT chunk, rhs = wvT)
  ST[k, q] = KT_h.T @ QT_h            (contraction over 64 head dims)
  exp:  E = exp(ST) * exp(biasT)      (ACT exp, DVE multiply; host sends
                                       exp(bias).T so the bias add inside
                                       the exp becomes a multiply)
  out2T[i(+1), q] = [V_h | 1].T @ E   (ones column gives softmax row-sums
                                       for free in the same matmul)
  OT = out2T[0:64] * (1/sum) , broadcast via a K=1 PE matmul
  yT[d, q] = Wo @ OT + bo
Host transposes yT back. Matmuls run as float32r (1 cycle/row at N=512).
"""

import numpy as np
import concourse.bacc as bacc
import concourse.mybir as mybir
import concourse.tile as tile
from concourse import bass_utils

HEADS = 8
DH = 64
D = 512
NQ = 512          # queries per core (Nq=1024 split in halves)
NK = 2048
KC = NK // 128    # 16 key chunks
SCALE = DH ** -0.5

F32 = mybir.dt.float32
F32R = mybir.dt.float32r
AF = mybir.ActivationFunctionType


def _build_nc():
    nc = bacc.Bacc("TRN2", target_bir_lowering=False, debug=False)

    xT_d = nc.dram_tensor("xT", [D, NQ], F32R, kind="ExternalInput")
    ctxT_d = nc.dram_tensor("ctxT", [D, NK], F32R, kind="ExternalInput")
    expB_d = nc.dram_tensor("expB", [NK, NQ], F32, kind="ExternalInput")
    wqT_d = nc.dram_tensor("wqT", [D, D], F32R, kind="ExternalInput")
    wkT_d = nc.dram_tensor("wkT", [D, D], F32R, kind="ExternalInput")
    wvT_d = nc.dram_tensor("wvT", [D, D], F32R, kind="ExternalInput")
    woT_d = nc.dram_tensor("woT", [D, D], F32R, kind="ExternalInput")
    bo_d = nc.dram_tensor("bo", [D, 1], F32, kind="ExternalInput")
    yT_d = nc.dram_tensor("yT", [D, NQ], F32, kind="ExternalOutput")

    with tile.TileContext(nc) as tc, nc.allow_low_precision(
            reason="float32r matmul operands (rounded fp32)"):
        with (
            tc.tile_pool(name="const", bufs=1) as const,
            tc.tile_pool(name="main", bufs=1) as main,
            tc.tile_pool(name="work", bufs=3) as work,
            tc.tile_pool(name="norm", bufs=2) as norm,
        ):
            # ---- load weights / activations ----
            wq = [const.tile([128, D], F32R, name=f"wq{i}", tag=f"wq{i}") for i in range(4)]
            wk = [const.tile([128, D], F32R, name=f"wk{i}", tag=f"wk{i}") for i in range(4)]
            wv = [const.tile([128, D], F32R, name=f"wv{i}", tag=f"wv{i}") for i in range(4)]
            wo = [const.tile([128, D], F32R, name=f"wo{i}", tag=f"wo{i}") for i in range(4)]
            bo_sb = [const.tile([128, 1], F32, name=f"bo{i}", tag=f"bo{i}") for i in range(4)]
            ones_sb = const.tile([1, DH], F32R, name="ones", tag="ones")
            onesF = const.tile([128, 1], F32, name="onesF", tag="onesF")
            nc.vector.memset(onesF, 1.0)
            nc.vector.tensor_copy(ones_sb, onesF[0:1, 0:1].broadcast_to([1, DH]))
            for i in range(4):
                sl = slice(i * 128, (i + 1) * 128)
                nc.sync.dma_start(out=wq[i], in_=wqT_d[sl, :])
                nc.sync.dma_start(out=wk[i], in_=wkT_d[sl, :])
                nc.sync.dma_start(out=wv[i], in_=wvT_d[sl, :])
                nc.sync.dma_start(out=wo[i], in_=woT_d[sl, :])
                nc.sync.dma_start(out=bo_sb[i], in_=bo_d[sl, :])

            KT = [main.tile([128, NK], F32R, name=f"KT{i}", tag=f"KT{i}") for i in range(4)]
            QT = [main.tile([128, NQ], F32R, name=f"QT{i}", tag=f"QT{i}") for i in range(4)]
            OT = [main.tile([128, NQ], F32R, name=f"OT{i}", tag=f"OT{i}") for i in range(4)]
            Vo = [main.tile([128, HEADS, DH + 1], F32R, name=f"Vo{c}", tag=f"Vo{c}")
                  for c in range(KC)]
            for c in range(KC):
                nc.vector.tensor_copy(
                    Vo[c][:, :, DH], onesF[:, 0:1].broadcast_to([128, HEADS]))

            with (
                tc.tile_pool(name="ctxp", bufs=1) as ctxp,
                tc.tile_pool(name="psA", bufs=4, space="PSUM") as psA,
            ):
                ctx = [ctxp.tile([128, NK], F32R, name=f"ctx{i}", tag=f"ctx{i}") for i in range(4)]
                xts = [ctxp.tile([128, NQ], F32R, name=f"xts{i}", tag=f"xts{i}") for i in range(4)]
                for i in range(4):
                    nc.sync.dma_start(out=ctx[i], in_=ctxT_d[i * 128:(i + 1) * 128, :])
                    nc.sync.dma_start(out=xts[i], in_=xT_d[i * 128:(i + 1) * 128, :])

                # ---- K projection: KT[mi][:, nt] = sum_ki wk[ki][:,mi].T @ ctx[ki][:,nt]
                for mi in range(4):
                    msl = slice(mi * 128, (mi + 1) * 128)
                    for nt in range(4):
                        nsl = slice(nt * 512, (nt + 1) * 512)
                        ps = psA.tile([128, 512], F32, name="proj", tag="proj")
                        for ki in range(4):
                            nc.tensor.matmul(
                                ps, (wk[ki][:, msl]), (ctx[ki][:, nsl]),
                                start=(ki == 0), stop=(ki == 3))
                        nc.any.tensor_copy(KT[mi][:, nsl], ps)

                # ---- V projection -> Vo[c][:, h, 0:64]
                for c in range(KC):
                    csl = slice(c * 128, (c + 1) * 128)
                    ps = psA.tile([128, 512], F32, name="proj", tag="proj")
                    for ki in range(4):
                        nc.tensor.matmul(
                            ps, (ctx[ki][:, csl]), (wv[ki]),
                            start=(ki == 0), stop=(ki == 3))
                    nc.any.tensor_copy(
                        Vo[c][:, :, 0:DH],
                        ps.rearrange("p (h d) -> p h d", h=HEADS))

                # ---- Q projection
                for mi in range(4):
                    msl = slice(mi * 128, (mi + 1) * 128)
                    ps = psA.tile([128, 512], F32, name="proj", tag="proj")
                    for ki in range(4):
                        nc.tensor.matmul(
                            ps, (wq[ki][:, msl]), (xts[ki]),
                            start=(ki == 0), stop=(ki == 3))
                    nc.any.tensor_copy(QT[mi], ps)

            # ---- attention ----
            with (
                tc.tile_pool(name="ebp", bufs=1) as ebp,
                tc.tile_pool(name="psS", bufs=3, space="PSUM") as psS,
                tc.tile_pool(name="psO", bufs=2, space="PSUM") as psO,
                tc.tile_pool(name="psM", bufs=1, space="PSUM") as psM,
                tc.tile_pool(name="psY", bufs=2, space="PSUM") as psY,
            ):
                eB = [ebp.tile([128, NQ], F32, name=f"eB{c}", tag=f"eB{c}") for c in range(KC)]
                for c in range(KC):
                    nc.sync.dma_start(out=eB[c], in_=expB_d[c * 128:(c + 1) * 128, :])
                for h in range(HEADS):
                    ti = h // 2
                    rsl = slice((h % 2) * DH, (h % 2) * DH + DH)
                    o2 = psO.tile([DH + 1, NQ], F32, name="o2", tag="o2")
                    for c in range(KC):
                        csl = slice(c * 128, (c + 1) * 128)
                        s = psS.tile([128, NQ], F32, name="s", tag="s")
                        nc.tensor.matmul(
                            s, (KT[ti][rsl, csl]), (QT[ti][rsl, :]),
                            start=True, stop=True)
                        e1 = work.tile([128, NQ], F32, name="e1", tag="e1")
                        nc.scalar.activation(e1, s, AF.Exp)
                        et = work.tile([128, NQ], F32R, name="et", tag="et")
                        nc.vector.tensor_mul(et, e1, eB[c])
                        nc.tensor.matmul(
                            o2, (Vo[c][:, h, :]), (et),
                            start=(c == 0), stop=(c == KC - 1))
                    # normalize: OT_h = o2[0:64] * (1/rowsum) (bcast over partitions)
                    ch = norm.tile([1, NQ], F32R, name="ch", tag="ch")
                    nc.vector.reciprocal(ch, o2[DH:DH + 1, :])
                    cb = psM.tile([DH, NQ], F32, name="cb", tag="cb")
                    nc.tensor.matmul(cb, (ones_sb), (ch), start=True, stop=True)
                    cbs = norm.tile([DH, NQ], F32, name="cbs", tag="cbs")
                    nc.any.tensor_copy(cbs, cb)
                    nc.vector.tensor_mul(OT[ti][rsl, :], o2[0:DH, :], cbs)

                # ---- output projection + bias ----
                for mi in range(4):
                    msl = slice(mi * 128, (mi + 1) * 128)
                    ps = psY.tile([128, NQ], F32, name="yTp", tag="yTp")
                    for ki in range(4):
                        nc.tensor.matmul(
                            ps, (wo[ki][:, msl]), (OT[ki]),
                            start=(ki == 0), stop=(ki == 3))
                    ysb = work.tile([128, NQ], F32, name="ysb", tag="ysb")
                    nc.vector.tensor_scalar_add(ysb, ps, bo_sb[mi])
                    nc.sync.dma_start(out=yT_d[msl, :], in_=ysb)

    nc.compile()
    return nc


_NC_CACHE = {}


def _get_nc():
    if "nc" not in _NC_CACHE:
        _NC_CACHE["nc"] = _build_nc()
    return _NC_CACHE["nc"]


def kernel(x, context, bias, Wq, Wk, Wv, Wo, bo):
    nc = _get_nc()
    x = np.asarray(x, dtype=np.float32)
    context = np.asarray(context, dtype=np.float32)
    bias = np.asarray(bias, dtype=np.float32)
    wqT = np.ascontiguousarray((np.asarray(Wq) * SCALE).T.astype(np.float32))
    wkT = np.ascontiguousarray(np.asarray(Wk).T.astype(np.float32))
    wvT = np.ascontiguousarray(np.asarray(Wv).T.astype(np.float32))
    woT = np.ascontiguousarray(np.asarray(Wo).T.astype(np.float32))
    bo2 = np.ascontiguousarray(np.asarray(bo, dtype=np.float32).reshape(D, 1))

    in_maps = []
    for core in range(8):
        b, half = core // 2, core % 2
        qs = half * NQ
        in_maps.append({
            "xT": np.ascontiguousarray(x[b, qs:qs + NQ, :].T),
            "ctxT": np.ascontiguousarray(context[b].T),
            "expB": np.ascontiguousarray(np.exp(bias[b, qs:qs + NQ, :]).T),
            "wqT": wqT, "wkT": wkT, "wvT": wvT, "woT": woT, "bo": bo2,
        })

    res = bass_utils.run_bass_kernel_spmd(
        nc, in_maps, core_ids=list(range(8)), trace=False)

    out = np.empty((4, 2 * NQ, D), dtype=np.float32)
    for core in range(8):
        b, half = core // 2, core % 2
        qs = half * NQ
        out[b, qs:qs + NQ, :] = res.results[core]["yT"].T
    return out


# revision 11
# speedup vs baseline: 1.3090x; 1.3090x over previous
"""Cross-attention kernel for Trainium2 (8 NeuronCores, SPMD).

Problem: B=4, Nq=1024, Nk=2048, D=512, 8 heads x 64 head-dim, fp32,
full-tensor bias added to scores before softmax.

Sharding: (batch, query-half) -> 8 disjoint shards, one per core. Each core
computes its own (512, 512) slice of the output; no collectives needed.
K/V projections are computed redundantly on the two cores sharing a batch.

Device layout: attention tensors kept transposed (feature/key dim on
partitions) so every matmul contraction lands on the partition axis:
  QT[d, q] = (SCALE*Wq) @ xT          KT[d, k] = Wk @ ctxT
  V[k, i]  = ctxT.T @ Wv.T
  ST[k, q] = KT_h.T @ QT_h            (contraction over the 64 head dims;
                                       the two heads of a pair sit in row
                                       groups 0-1/2-3 of the PE array and
                                       run concurrently)
  E = exp(ST) * exp(biasT - 4)        (ACT exp over a 2-bank pair tile,
                                       DVE/GPSIMD multiply; host sends
                                       exp(bias - 4).T so the bias add
                                       becomes a multiply and fp16 cannot
                                       overflow; the -4 cancels in the
                                       softmax normalization)
  out2T[i(+1), q] = [V_h | 1].T @ E   (ones column yields softmax row-sums
                                       in the same accumulation)
  OT = out2T[0:64] * recip(sum), broadcast across partitions via a K=1 matmul
  yT[d, q] = Wo @ OT + bo
Host transposes yT back. Matmul operands are fp16 (fp32 PSUM accumulate);
fp32r runs the PE at half clock (no HAM warm-up), fp16 does not.
"""

import numpy as np
import concourse.bass as bass
import concourse.bacc as bacc
import concourse.mybir as mybir
import concourse.tile as tile
from concourse import bass_utils

HEADS = 8
DH = 64
D = 512
NQ = 512          # queries per core (Nq=1024 split in halves)
NK = 2048
KC = NK // 128    # 16 key chunks
SCALE = DH ** -0.5
BSHIFT = 4.0      # exp(bias - BSHIFT): keeps fp16 weights in range

F32 = mybir.dt.float32
F16 = mybir.dt.float16
AF = mybir.ActivationFunctionType


def _bcast2(ap, n):
    """[128, F] -> [128, n, F] with a step-0 middle dim."""
    return bass.AP(ap.tensor, ap.offset, [ap.ap[0], [0, n], ap.ap[1]])


def _build_nc():
    nc = bacc.Bacc("TRN2", target_bir_lowering=False, debug=False)

    xT_d = nc.dram_tensor("xT", [D, NQ], F16, kind="ExternalInput")
    ctxT_d = nc.dram_tensor("ctxT", [D, NK], F16, kind="ExternalInput")
    expB_d = nc.dram_tensor("expB", [NK, NQ], F16, kind="ExternalInput")
    wqT_d = nc.dram_tensor("wqT", [D, D], F16, kind="ExternalInput")
    wkT_d = nc.dram_tensor("wkT", [D, D], F16, kind="ExternalInput")
    wvT_d = nc.dram_tensor("wvT", [D, D], F16, kind="ExternalInput")
    woT_d = nc.dram_tensor("woT", [D, D], F16, kind="ExternalInput")
    bo_d = nc.dram_tensor("bo", [D, 1], F32, kind="ExternalInput")
    yT_d = nc.dram_tensor("yT", [D, NQ], F32, kind="ExternalOutput")

    with tile.TileContext(nc) as tc, nc.allow_low_precision(
            reason="fp16 matmul operands, fp32 accumulation"):
        with (
            tc.tile_pool(name="const", bufs=1) as const,
            tc.tile_pool(name="main", bufs=1) as main,
            tc.tile_pool(name="work", bufs=3) as work,
            tc.tile_pool(name="norm", bufs=2) as norm,
        ):
            wq = [const.tile([128, D], F16, name=f"wq{i}", tag=f"wq{i}") for i in range(4)]
            wk = [const.tile([128, D], F16, name=f"wk{i}", tag=f"wk{i}") for i in range(4)]
            wv = [const.tile([128, D], F16, name=f"wv{i}", tag=f"wv{i}") for i in range(4)]
            wo = [const.tile([128, D], F16, name=f"wo{i}", tag=f"wo{i}") for i in range(4)]
            bo_sb = [const.tile([128, 1], F32, name=f"bo{i}", tag=f"bo{i}") for i in range(4)]
            ones_sb = const.tile([1, DH], F16, name="ones", tag="ones")
            onesF = const.tile([128, 1], F32, name="onesF", tag="onesF")
            nc.vector.memset(onesF, 1.0)
            nc.vector.tensor_copy(ones_sb, onesF[0:1, 0:1].broadcast_to([1, DH]))
            for i in range(4):
                sl = slice(i * 128, (i + 1) * 128)
                nc.sync.dma_start(out=wq[i], in_=wqT_d[sl, :])
                nc.sync.dma_start(out=wk[i], in_=wkT_d[sl, :])
                nc.sync.dma_start(out=wv[i], in_=wvT_d[sl, :])
                nc.sync.dma_start(out=wo[i], in_=woT_d[sl, :])
                nc.sync.dma_start(out=bo_sb[i], in_=bo_d[sl, :])

            KT = [main.tile([128, NK], F16, name=f"KT{i}", tag=f"KT{i}") for i in range(4)]
            QT = [main.tile([128, NQ], F16, name=f"QT{i}", tag=f"QT{i}") for i in range(4)]
            OT = [main.tile([128, NQ], F16, name=f"OT{i}", tag=f"OT{i}") for i in range(4)]
            Vo = [main.tile([128, HEADS, DH + 1], F16, name=f"Vo{c}", tag=f"Vo{c}")
                  for c in range(KC)]
            eB = [main.tile([128, NQ], F16, name=f"eB{c}", tag=f"eB{c}") for c in range(KC)]
            for c in range(KC):
                nc.vector.tensor_copy(
                    Vo[c][:, :, DH], onesF[:, 0:1].broadcast_to([128, HEADS]))
                nc.sync.dma_start(out=eB[c], in_=expB_d[c * 128:(c + 1) * 128, :])

            with (
                tc.tile_pool(name="ctxp", bufs=1) as ctxp,
                tc.tile_pool(name="psA", bufs=4, space="PSUM") as psA,
            ):
                ctx = [ctxp.tile([128, NK], F16, name=f"ctx{i}", tag=f"ctx{i}") for i in range(4)]
                xts = [ctxp.tile([128, NQ], F16, name=f"xts{i}", tag=f"xts{i}") for i in range(4)]
                for i in range(4):
                    nc.sync.dma_start(out=ctx[i], in_=ctxT_d[i * 128:(i + 1) * 128, :])
                    nc.sync.dma_start(out=xts[i], in_=xT_d[i * 128:(i + 1) * 128, :])

                # K projection
                for mi in range(4):
                    msl = slice(mi * 128, (mi + 1) * 128)
                    for nt in range(4):
                        nsl = slice(nt * 512, (nt + 1) * 512)
                        ps = psA.tile([128, 512], F32, name="proj", tag="proj")
                        for ki in range(4):
                            nc.tensor.matmul(
                                ps, wk[ki][:, msl], ctx[ki][:, nsl],
                                start=(ki == 0), stop=(ki == 3))
                        nc.any.tensor_copy(KT[mi][:, nsl], ps)

                # V projection -> Vo[c][:, h, 0:64]
                for c in range(KC):
                    csl = slice(c * 128, (c + 1) * 128)
                    ps = psA.tile([128, 512], F32, name="proj", tag="proj")
                    for ki in range(4):
                        nc.tensor.matmul(
                            ps, ctx[ki][:, csl], wv[ki],
                            start=(ki == 0), stop=(ki == 3))
                    nc.any.tensor_copy(
                        Vo[c][:, :, 0:DH],
                        ps.rearrange("p (h d) -> p h d", h=HEADS))

                # Q projection
                for mi in range(4):
                    msl = slice(mi * 128, (mi + 1) * 128)
                    ps = psA.tile([128, 512], F32, name="proj", tag="proj")
                    for ki in range(4):
                        nc.tensor.matmul(
                            ps, wq[ki][:, msl], xts[ki],
                            start=(ki == 0), stop=(ki == 3))
                    nc.any.tensor_copy(QT[mi], ps)

            # ---- attention (head pairs) ----
            with (
                tc.tile_pool(name="psS", bufs=2, space="PSUM") as psS,
                tc.tile_pool(name="psO", bufs=3, space="PSUM") as psO,
                tc.tile_pool(name="psM", bufs=1, space="PSUM") as psM,
            ):
                for hp in range(4):
                    h0, h1 = 2 * hp, 2 * hp + 1
                    lo, hi = slice(0, DH), slice(DH, 128)
                    o2a = psO.tile([DH + 1, NQ], F32, name="o2a", tag="o2")
                    o2b = psO.tile([DH + 1, NQ], F32, name="o2b", tag="o2")
                    for c in range(KC):
                        csl = slice(c * 128, (c + 1) * 128)
                        s = psS.tile([128, 2, NQ], F32, name="s", tag="s")
                        nc.tensor.matmul(
                            s[:, 0, :], KT[hp][lo, csl], QT[hp][lo, :],
                            start=True, stop=True)
                        nc.tensor.matmul(
                            s[:, 1, :], KT[hp][hi, csl], QT[hp][hi, :],
                            start=True, stop=True)
                        e1 = work.tile([128, 2, NQ], F16, name="e1", tag="e1")
                        nc.scalar.activation(e1, s, AF.Exp)
                        et = work.tile([128, 2, NQ], F16, name="et", tag="et")
                        eng = nc.gpsimd if (c % 4 == 3) else nc.vector
                        eng.tensor_mul(et, e1, _bcast2(eB[c], 2))
                        nc.tensor.matmul(
                            o2a, Vo[c][:, h0, :], et[:, 0, :],
                            start=(c == 0), stop=(c == KC - 1))
                        nc.tensor.matmul(
                            o2b, Vo[c][:, h1, :], et[:, 1, :],
                            start=(c == 0), stop=(c == KC - 1))
                    # normalize each head of the pair
                    for h, o2 in ((h0, o2a), (h1, o2b)):
                        rsl = slice((h % 2) * DH, (h % 2) * DH + DH)
                        ch = norm.tile([1, NQ], F16, name="ch", tag="ch")
                        nc.vector.reciprocal(ch, o2[DH:DH + 1, :])
                        cb = psM.tile([DH, NQ], F32, name="cb", tag="cb")
                        nc.tensor.matmul(cb, ones_sb, ch, start=True, stop=True)
                        cbs = norm.tile([DH, NQ], F16, name="cbs", tag="cbs")
                        nc.scalar.copy(cbs, cb)
                        nc.vector.tensor_mul(OT[hp][rsl, :], o2[0:DH, :], cbs)

            # ---- output projection + bias ----
            with tc.tile_pool(name="psY", bufs=2, space="PSUM") as psY:
                for mi in range(4):
                    msl = slice(mi * 128, (mi + 1) * 128)
                    ps = psY.tile([128, NQ], F32, name="yTp", tag="yTp")
                    for ki in range(4):
                        nc.tensor.matmul(
                            ps, wo[ki][:, msl], OT[ki],
                            start=(ki == 0), stop=(ki == 3))
                    ysb = work.tile([128, NQ], F32, name="ysb", tag="ysb")
                    nc.scalar.activation(ysb, ps, AF.Identity, bias=bo_sb[mi])
                    nc.sync.dma_start(out=yT_d[msl, :], in_=ysb)

    nc.compile()
    return nc


_NC_CACHE = {}


def _get_nc():
    if "nc" not in _NC_CACHE:
        _NC_CACHE["nc"] = _build_nc()
    return _NC_CACHE["nc"]


def make_in_maps(x, context, bias, Wq, Wk, Wv, Wo, bo):
    x = np.asarray(x, dtype=np.float32)
    context = np.asarray(context, dtype=np.float32)
    bias = np.asarray(bias, dtype=np.float32)
    wqT = np.ascontiguousarray((np.asarray(Wq) * SCALE).T).astype(np.float16)
    wkT = np.ascontiguousarray(np.asarray(Wk).T).astype(np.float16)
    wvT = np.ascontiguousarray(np.asarray(Wv).T).astype(np.float16)
    woT = np.ascontiguousarray(np.asarray(Wo).T).astype(np.float16)
    bo2 = np.ascontiguousarray(np.asarray(bo, dtype=np.float32).reshape(D, 1))

    in_maps = []
    for core in range(8):
        b, half = core // 2, core % 2
        qs = half * NQ
        in_maps.append({
            "xT": np.ascontiguousarray(x[b, qs:qs + NQ, :].T).astype(np.float16),
            "ctxT": np.ascontiguousarray(context[b].T).astype(np.float16),
            "expB": np.ascontiguousarray(
                np.exp(bias[b, qs:qs + NQ, :] - BSHIFT).T).astype(np.float16),
            "wqT": wqT, "wkT": wkT, "wvT": wvT, "woT": woT, "bo": bo2,
        })
    return in_maps


def kernel(x, context, bias, Wq, Wk, Wv, Wo, bo):
    nc = _get_nc()
    in_maps = make_in_maps(x, context, bias, Wq, Wk, Wv, Wo, bo)
    res = bass_utils.run_bass_kernel_spmd(
        nc, in_maps, core_ids=list(range(8)), trace=False)

    out = np.empty((4, 2 * NQ, D), dtype=np.float32)
    for core in range(8):
        b, half = core // 2, core % 2
        qs = half * NQ
        out[b, qs:qs + NQ, :] = res.results[core]["yT"].T
    return out
